# revision 13
# baseline (speedup 1.0000x reference)
"""Trainium2 Bass kernel for a correlation-corrected cross-entropy loss.

Math (per batch row i of logits[B, C], with t = target[i]):
    S_i   = sum_c exp(logits[i, c])            (no max-shift needed: inputs ~N(0,1))
    p_t   = exp(logits[i, t]) / S_i
    P1    = exp(logits[i, Y1[t]]) / S_i
    P2    = exp(logits[i, Y2[t]]) / S_i
    corr  = T * (X1[t] * P1 + X2[t] * P2)
    cond  = p_t > corr
    loss_i = -log(p_t - corr) if cond else -log(p_t)
    k_i   = cond and (P1 != 0 or P2 != 0)
    z_i   = p_t / corr if k_i else 0
    j_i   = not cond
Outputs: (sum(loss_i)/B, sum(k_i), sum(z_i), sum(j_i)).

Sharding: data-parallel over the batch dim across 8 NeuronCores (512 rows
each). The [1, C] lookup tables and T are replicated to every core. Each
core reduces its 512 rows to 4 partial scalars on-device; the host sums the
8 partials (the "all-reduce") and applies the 1/B scale and the loss
negation.

Per-core kernel: stream the [512, 32000] f32 logits shard through SBUF in
[128, W] tiles; ScalarE computes exp in place with fused row-sum
accumulation (activation accum_out), so each element is touched by exactly
one DMA load and one ACT pass -> memory-bound. The per-row gathers (tables
via target, logits at 3 data-dependent columns) are done with GPSIMD
indirect DMAs and fully hidden under the stream.

Schedule notes (from perfetto traces of earlier revisions):
  * All stream DMAs ride the single Sync HWDGE ring, in column order. A
    two-ring variant (head tiles on the Scalar ring) regressed: the SDMA
    engines round-robin between rings at packet granularity, so the head
    tiles completed ~2x late and the in-order ACT queue stalled.
  * The gather-dependent small-exp chain is EMITTED between group 2 and
    group 3 so the in-order ACT queue never stalls on the slow indirect
    gathers (previously a 25us stall that left ACT running a backlog to
    3.3us past the last stream byte).
  * Group 3's widths taper gradually (see TAPER) so ACT's one-tile lag
    drains to ~1.3us by the last byte, and the post-stream chain (S3 add,
    ln, loss combine) runs entirely on ACT via Identity-with-bias tricks
    -- no DVE round trips before the output DMA.
"""

import numpy as np

import concourse.bacc as bacc
import concourse.bass as bass
import concourse.mybir as mybir
import concourse.tile as tile
from concourse.bass import IndirectOffsetOnAxis
from concourse.bass_utils import run_bass_kernel_spmd

B, C = 4096, 32000
NCORES = 8
R = B // NCORES          # rows per core: 512
P = 128                  # SBUF partitions
G = R // P               # row groups per core: 4
W = 8000                 # streaming column-tile width

# Group 3 tapers gradually: the in-order ACT engine trails the DMA stream
# by one full-tile EXP (~7.2us); an abrupt taper lets DMA sprint through
# the small tiles while ACT drains that backlog, so ACT (not DMA) would end
# the kernel. This profile (from a calibrated lag-recurrence search) hands
# ACT steadily shrinking tiles so it finishes ~1.3us after the last byte.
FULL = [W, W, W, W]                                        # groups 0-2
TAPER = [7625, 6000, 4250, 2375, 2250, 2125, 2125, 1500, 1500, 1250, 1000]
WIDTHS = [FULL, FULL, FULL, TAPER]
assert all(sum(ws) == C for ws in WIDTHS)
NCOLS = sum(len(ws) for ws in WIDTHS)

f32 = mybir.dt.float32
i32 = mybir.dt.int32
Alu = mybir.AluOpType
Act = mybir.ActivationFunctionType
AX = mybir.AxisListType.X


def _build_kernel() -> bass.Bass:
    nc = bacc.Bacc()
    x = nc.declare_dram_parameter("x", [R, C], f32, isOutput=False)
    tgt = nc.declare_dram_parameter("tgt", [P, G], i32, isOutput=False)
    tblf = nc.declare_dram_parameter("tblf", [C, 2], f32, isOutput=False)  # X1|X2
    tbli = nc.declare_dram_parameter("tbli", [C, 2], i32, isOutput=False)  # Y1|Y2
    tval = nc.declare_dram_parameter("tval", [P, 1], f32, isOutput=False)
    out = nc.declare_dram_parameter("out", [P, 4], f32, isOutput=True)

    with tile.TileContext(nc) as tc:
        _kernel_body(tc, x, tgt, tblf, tbli, tval, out)
    nc.compile()
    _merge_act_table_loads(nc)
    return nc


def _merge_act_table_loads(nc):
    """The auto-inserted ACT table loads pick exp_and_others then
    natural_log, paying a ~2.7us table switch right in the kernel tail.
    Set 6 (natural_log_exp_and_others) contains Exp, Ln, Identity and Copy,
    so point the first load at it and drop the later ones (they carry no
    sync)."""
    loads = [
        inst
        for f in nc.m.functions
        for blk in f.blocks
        for inst in blk.instructions
        if isinstance(inst, mybir.InstLoadActFuncSet)
    ]
    if any(inst.sync_info is not None for inst in loads):
        return  # unexpected shape; leave the program untouched
    first = True
    for f in nc.m.functions:
        for blk in f.blocks:
            keep = []
            for inst in blk.instructions:
                if isinstance(inst, mybir.InstLoadActFuncSet):
                    if first:
                        inst.act_func_set_id = 6
                        first = False
                    else:
                        continue
                keep.append(inst)
            if len(keep) != len(blk.instructions):
                blk.instructions[:] = keep


def _kernel_body(tc, x, tgt, tblf, tbli, tval, out):
    nc = tc.nc
    with (
        tc.tile_pool(name="const", bufs=1) as const,
        tc.tile_pool(name="stream", bufs=6) as stream,
        tc.tile_pool(name="small", bufs=1) as small,
    ):
        stats = const.tile([P, NCOLS], f32)
        # Explicit zero-bias tile for every activation: a float bias would
        # force a const-AP tensor whose TENSOR_LOAD sits on the Sync queue
        # ahead of the first stream DMA (~1.2us of dead start time). Its
        # memset must be EMITTED before the first EXP that reads it (the
        # dependency tracker orders by emission, a later write would be WAR).
        zbias = const.tile([P, 1], f32)
        nc.vector.memset(zbias[:], 0.0)
        ones = const.tile([P, G], f32)
        nc.vector.memset(ones[:], 1.0)

        def stream_tile(g, coff, w, col):
            xt = stream.tile([P, W], f32, tag="xt")
            nc.sync.dma_start(out=xt[:, :w], in_=x[g * P:(g + 1) * P, coff:coff + w])
            # exp in place; only the fused row-sum (accum_out) is kept.
            nc.scalar.activation(
                out=xt[:, :w], in_=xt[:, :w], func=Act.Exp,
                bias=zbias[:, 0:1],
                accum_out=stats[:, col:col + 1])

        col_of = {}
        col = 0
        for g in range(G):
            for ti in range(len(WIDTHS[g])):
                col_of[(g, ti)] = col
                col += 1

        # ---- head: first two stream tiles before the small loads so the
        # memory-bound stream is first in the Sync queue -------------------
        coff = 0
        for ti in range(2):
            stream_tile(0, coff, WIDTHS[0][ti], col_of[(0, ti)])
            coff += WIDTHS[0][ti]

        # ---- small loads (on the GPSIMD DMA queue so they don't delay the
        # stream DMAs queued on Sync) ---------------------------------------
        t_tile = const.tile([P, G], i32)      # t_tile[p, g] = target[g*128 + p]
        nc.gpsimd.dma_start(out=t_tile[:], in_=tgt[:, :])
        tv = const.tile([P, 1], f32)
        nc.gpsimd.dma_start(out=tv[:], in_=tval[:, :])

        # ---- table gathers: row t of [C, 2] tables, per batch row --------
        # HW indirect DMA honors one offset per partition, so gather each
        # row-group (one [P, 1] offset column) separately.
        x1 = small.tile([P, G], f32)
        x2 = small.tile([P, G], f32)
        y1 = small.tile([P, G], i32)
        y2 = small.tile([P, G], i32)
        for g in range(G):
            xg = small.tile([P, 2], f32, tag=f"xg{g}")  # (X1[t], X2[t])
            nc.gpsimd.indirect_dma_start(
                out=xg[:], out_offset=None, in_=tblf[:, :],
                in_offset=IndirectOffsetOnAxis(ap=t_tile[:, g:g + 1], axis=0),
            )
            yg = small.tile([P, 2], i32, tag=f"yg{g}")  # (Y1[t], Y2[t])
            nc.gpsimd.indirect_dma_start(
                out=yg[:], out_offset=None, in_=tbli[:, :],
                in_offset=IndirectOffsetOnAxis(ap=t_tile[:, g:g + 1], axis=0),
            )
            nc.vector.tensor_copy(out=x1[:, g:g + 1], in_=xg[:, 0:1])
            nc.vector.tensor_copy(out=x2[:, g:g + 1], in_=xg[:, 1:2])
            nc.vector.tensor_copy(out=y1[:, g:g + 1], in_=yg[:, 0:1])
            nc.vector.tensor_copy(out=y2[:, g:g + 1], in_=yg[:, 1:2])

        # ---- flat element offsets into x for the 3 logit gathers ---------
        ridx = const.tile([P, G], i32)        # ridx[p, g] = g*128 + p
        nc.gpsimd.iota(out=ridx[:], pattern=[[P, G]], base=0,
                       channel_multiplier=1)
        rb = const.tile([P, G], i32)          # rb[p, g] = (g*128 + p) * C
        nc.vector.tensor_scalar(out=rb[:], in0=ridx[:], scalar1=C,
                                scalar2=None, op0=Alu.mult)
        off_t = small.tile([P, G], i32)
        nc.vector.tensor_tensor(out=off_t[:], in0=rb[:], in1=t_tile[:], op=Alu.add)
        off_1 = small.tile([P, G], i32)
        nc.vector.tensor_tensor(out=off_1[:], in0=rb[:], in1=y1[:], op=Alu.add)
        off_2 = small.tile([P, G], i32)
        nc.vector.tensor_tensor(out=off_2[:], in0=rb[:], in1=y2[:], op=Alu.add)

        xap = x[:, :]
        xflat = bass.AP(tensor=xap.tensor, offset=0, ap=[[1, R * C], [1, 1]])
        g_t = small.tile([P, G], f32)         # logits[i, t]
        g_1 = small.tile([P, G], f32)         # logits[i, Y1[t]]
        g_2 = small.tile([P, G], f32)         # logits[i, Y2[t]]
        for g in range(G):
            for dst, off in ((g_t, off_t), (g_1, off_1), (g_2, off_2)):
                nc.gpsimd.indirect_dma_start(
                    out=dst[:, g:g + 1], out_offset=None, in_=xflat,
                    in_offset=IndirectOffsetOnAxis(ap=off[:, g:g + 1], axis=0),
                )

        # ---- groups 0-2 of the stream ------------------------------------
        S = small.tile([P, G], f32)           # S[p, g] = row sum of exp
        lnS = small.tile([P, G], f32)

        def emit_group(g, skip=0):
            coff = sum(WIDTHS[g][:skip])
            for ti in range(skip, len(WIDTHS[g])):
                stream_tile(g, coff, WIDTHS[g][ti], col_of[(g, ti)])
                coff += WIDTHS[g][ti]

        def reduce_group(g):
            c0 = col_of[(g, 0)]
            c1 = col_of[(g, len(WIDTHS[g]) - 1)] + 1
            c = slice(g, g + 1)
            nc.vector.tensor_reduce(
                out=S[:, c], in_=stats[:, c0:c1], axis=AX, op=Alu.add)
            nc.scalar.activation(out=lnS[:, c], in_=S[:, c], func=Act.Ln,
                                 bias=zbias[:, 0:1])

        emit_group(0, skip=2)
        reduce_group(0)
        emit_group(1)
        reduce_group(1)
        emit_group(2)
        reduce_group(2)

        # ---- S-independent per-row math ----------------------------------
        # Emitted AFTER group 2 so the ACT-queue position of these small
        # exps is far past the point where the indirect gathers complete:
        # the in-order ACT queue never stalls on them.
        # The row-sum S only scales p_t/P1/P2 uniformly, so every
        # comparison and ratio can be computed from the raw exp'd logits:
        #   cond:  p_t > corr      <=>  e_t > cnum,  cnum = T*(x1*e1 + x2*e2)
        #   z:     p_t / corr       =   e_t / cnum
        #   nz:    P1 != 0 or P2 != 0  <=>  e_1 != 0 or e_2 != 0
        # Only the loss term needs S, and it splits as
        #   -log(d_pre / S) = log(S) - log(d_pre),
        #   d_pre = (e_t - cnum) if cond else e_t,
        # so log(d_pre) is computed early too; only log(S) trails the
        # stream.
        e_t = small.tile([P, G], f32)
        nc.scalar.activation(out=e_t[:], in_=g_t[:], func=Act.Exp, bias=zbias[:, 0:1])
        e_1 = small.tile([P, G], f32)
        nc.scalar.activation(out=e_1[:], in_=g_1[:], func=Act.Exp, bias=zbias[:, 0:1])
        e_2 = small.tile([P, G], f32)
        nc.scalar.activation(out=e_2[:], in_=g_2[:], func=Act.Exp, bias=zbias[:, 0:1])
        a = small.tile([P, G], f32)
        nc.vector.tensor_tensor(out=a[:], in0=x1[:], in1=e_1[:], op=Alu.mult)
        b = small.tile([P, G], f32)
        nc.vector.tensor_tensor(out=b[:], in0=x2[:], in1=e_2[:], op=Alu.mult)
        s = small.tile([P, G], f32)
        nc.vector.tensor_tensor(out=s[:], in0=a[:], in1=b[:], op=Alu.add)
        cnum = small.tile([P, G], f32)        # corr * S
        nc.vector.tensor_scalar(out=cnum[:], in0=s[:], scalar1=tv[:, 0:1],
                                scalar2=None, op0=Alu.mult)
        cond_i = small.tile([P, G], i32)      # 1 where p_t > corr (int mask)
        nc.vector.tensor_tensor(out=cond_i[:], in0=e_t[:], in1=cnum[:], op=Alu.is_gt)
        cond = small.tile([P, G], f32)
        nc.vector.tensor_copy(out=cond[:], in_=cond_i[:])
        diff = small.tile([P, G], f32)
        nc.vector.tensor_tensor(out=diff[:], in0=e_t[:], in1=cnum[:], op=Alu.subtract)
        d_pre = small.tile([P, G], f32)
        nc.vector.select(out=d_pre[:], mask=cond_i[:], on_true=diff[:], on_false=e_t[:])
        nz1 = small.tile([P, G], i32)
        nc.vector.tensor_scalar(out=nz1[:], in0=e_1[:], scalar1=0.0,
                                scalar2=None, op0=Alu.not_equal)
        nz2 = small.tile([P, G], i32)
        nc.vector.tensor_scalar(out=nz2[:], in0=e_2[:], scalar1=0.0,
                                scalar2=None, op0=Alu.not_equal)
        nz = small.tile([P, G], i32)
        nc.vector.tensor_tensor(out=nz[:], in0=nz1[:], in1=nz2[:], op=Alu.bitwise_or)
        k_i = small.tile([P, G], i32)         # cond and nz (int mask)
        nc.vector.tensor_tensor(out=k_i[:], in0=cond_i[:], in1=nz[:], op=Alu.bitwise_and)
        k = small.tile([P, G], f32)
        nc.vector.tensor_copy(out=k[:], in_=k_i[:])
        safe = small.tile([P, G], f32)        # cnum where k else 1.0
        nc.vector.select(out=safe[:], mask=k_i[:], on_true=cnum[:], on_false=ones[:])
        rsafe = small.tile([P, G], f32)
        nc.vector.reciprocal(out=rsafe[:], in_=safe[:])
        z0 = small.tile([P, G], f32)
        nc.vector.tensor_tensor(out=z0[:], in0=e_t[:], in1=rsafe[:], op=Alu.mult)
        z = small.tile([P, G], f32)
        nc.vector.tensor_tensor(out=z[:], in0=z0[:], in1=k[:], op=Alu.mult)
        j = small.tile([P, G], f32)           # 1 - cond
        nc.vector.tensor_scalar(out=j[:], in0=cond[:], scalar1=-1.0,
                                scalar2=1.0, op0=Alu.mult, op1=Alu.add)
        lnd_pre = small.tile([P, G], f32)
        nc.scalar.activation(out=lnd_pre[:], in_=d_pre[:], func=Act.Ln, bias=zbias[:, 0:1])
        Qd = small.tile([P, 1], f32)          # per-partition sum of ln(d_pre)
        nc.vector.tensor_reduce(out=Qd[:], in_=lnd_pre[:], axis=AX, op=Alu.add)
        Q = small.tile([P, 4], f32)
        nc.vector.tensor_reduce(out=Q[:, 1:2], in_=k[:], axis=AX, op=Alu.add)
        nc.vector.tensor_reduce(out=Q[:, 2:3], in_=z[:], axis=AX, op=Alu.add)
        nc.vector.tensor_reduce(out=Q[:, 3:4], in_=j[:], axis=AX, op=Alu.add)
        # fold groups 0-2's ln(S) into the running loss partial
        t01 = small.tile([P, 1], f32)         # lnS0 + lnS1
        nc.vector.tensor_tensor(out=t01[:], in0=lnS[:, 0:1], in1=lnS[:, 1:2],
                                op=Alu.add)
        t012 = small.tile([P, 1], f32)
        nc.vector.tensor_tensor(out=t012[:], in0=t01[:], in1=lnS[:, 2:3],
                                op=Alu.add)
        Qpre = small.tile([P, 1], f32)        # Qd - (lnS0 + lnS1 + lnS2)
        nc.vector.tensor_tensor(out=Qpre[:], in0=Qd[:], in1=t012[:],
                                op=Alu.subtract)

        # ---- group 3 (tapered) and the all-ACT tail ----------------------
        n3 = len(WIDTHS[3])
        coff = 0
        for ti in range(n3 - 1):
            stream_tile(3, coff, WIDTHS[3][ti], col_of[(3, ti)])
            coff += WIDTHS[3][ti]
        c0 = col_of[(3, 0)]
        clast = col_of[(3, n3 - 1)]
        S3a = small.tile([P, 1], f32)         # partial S3 over all but last tile
        nc.vector.tensor_reduce(out=S3a[:], in_=stats[:, c0:clast], axis=AX,
                                op=Alu.add)
        stream_tile(3, coff, WIDTHS[3][n3 - 1], clast)
        # Tail runs entirely on ACT (Identity-with-AP-bias), no DVE hops:
        #   S3 = stats[last] + S3a; lnS3 = ln(S3); Q0 = Qpre - lnS3.
        nc.scalar.activation(out=S[:, 3:4], in_=stats[:, clast:clast + 1],
                             func=Act.Identity, bias=S3a[:, 0:1])
        nc.scalar.activation(out=lnS[:, 3:4], in_=S[:, 3:4], func=Act.Ln,
                             bias=zbias[:, 0:1])
        nc.scalar.activation(out=Q[:, 0:1], in_=lnS[:, 3:4],
                             func=Act.Identity, bias=Qpre[:, 0:1], scale=-1.0)

        # ---- per-partition partials out; host sums the 128 lanes ---------
        nc.sync.dma_start(out=out[:, :], in_=Q[:])


_NC_CACHE = None


def _get_nc() -> bass.Bass:
    global _NC_CACHE
    if _NC_CACHE is None:
        _NC_CACHE = _build_kernel()
    return _NC_CACHE


def make_in_maps(input, target, X1, Y1, X2, Y2, T):
    """Shard the full inputs into per-core input maps."""
    input = np.ascontiguousarray(np.asarray(input, dtype=np.float32))
    target = np.asarray(target).astype(np.int32)
    tblf = np.ascontiguousarray(
        np.stack([np.asarray(X1, np.float32)[0], np.asarray(X2, np.float32)[0]],
                 axis=1))
    tbli = np.ascontiguousarray(
        np.stack([np.asarray(Y1)[0].astype(np.int32),
                  np.asarray(Y2)[0].astype(np.int32)], axis=1))
    tval = np.full((P, 1), np.asarray(T, np.float32)[0], dtype=np.float32)

    in_maps = []
    for c in range(NCORES):
        tg = target[c * R:(c + 1) * R].reshape(G, P).T  # [P, G]
        in_maps.append({
            "x": np.ascontiguousarray(input[c * R:(c + 1) * R]),
            "tgt": np.ascontiguousarray(tg),
            "tblf": tblf,
            "tbli": tbli,
            "tval": tval,
        })
    return in_maps


def combine_outputs(results):
    """Sum the per-core, per-partition [128, 4] partials on the host."""
    outs = np.stack([np.asarray(r["out"]) for r in results])  # [ncores, P, 4]
    tot = outs.sum(axis=(0, 1), dtype=np.float64)
    loss = np.float32(-tot[0] / B)
    return (loss, np.float32(tot[1]), np.float32(tot[2]), np.float32(tot[3]))


def kernel(input, target, X1, Y1, X2, Y2, T):
    nc = _get_nc()
    in_maps = make_in_maps(input, target, X1, Y1, X2, Y2, T)
    res = run_bass_kernel_spmd(nc, in_maps, core_ids=list(range(NCORES)))
    return combine_outputs(res.results)


# revision 17
# speedup vs baseline: 1.0950x; 1.0950x over previous
"""Trainium2 Bass kernel for a correlation-corrected cross-entropy loss.

Math (per batch row i of logits[B, C], with t = target[i]):
    S_i   = sum_c exp(logits[i, c])            (no max-shift needed: inputs ~N(0,1))
    p_t   = exp(logits[i, t]) / S_i
    P1    = exp(logits[i, Y1[t]]) / S_i
    P2    = exp(logits[i, Y2[t]]) / S_i
    corr  = T * (X1[t] * P1 + X2[t] * P2)
    cond  = p_t > corr
    loss_i = -log(p_t - corr) if cond else -log(p_t)
    k_i   = cond and (P1 != 0 or P2 != 0)
    z_i   = p_t / corr if k_i else 0
    j_i   = not cond
Outputs: (sum(loss_i)/B, sum(k_i), sum(z_i), sum(j_i)).

Sharding: data-parallel over the batch dim across 8 NeuronCores (512 rows
each). The [1, C] lookup tables and T are replicated to every core. Each
core reduces its 512 rows to 4 partial scalars on-device; the host sums the
8 partials (the "all-reduce") and applies the 1/B scale and the loss
negation.

Per-core kernel: stream the [512, 32000] f32 logits shard through SBUF in
[128, W] tiles; ScalarE computes exp in place with fused row-sum
accumulation (activation accum_out), so each element is touched by exactly
one DMA load and one ACT pass -> memory-bound. The per-row gathers (tables
via target, logits at 3 data-dependent columns) are done with GPSIMD
indirect DMAs and fully hidden under the stream.

Schedule notes (from perfetto traces of earlier revisions):
  * All stream DMAs ride the single Sync HWDGE ring, in column order. A
    two-ring variant (head tiles on the Scalar ring) regressed: the SDMA
    engines round-robin between rings at packet granularity, so the head
    tiles completed ~2x late and the in-order ACT queue stalled.
  * The gather-dependent small-exp chain is EMITTED between group 2 and
    group 3 so the in-order ACT queue never stalls on the slow indirect
    gathers (previously a 25us stall that left ACT running a backlog to
    3.3us past the last stream byte).
  * Group 3's widths taper gradually (see TAPER) so ACT's one-tile lag
    drains to ~1.3us by the last byte, and the post-stream chain (S3 add,
    ln, loss combine) runs entirely on ACT via Identity-with-bias tricks
    -- no DVE round trips before the output DMA.
"""

import numpy as np

import concourse.bacc as bacc
import concourse.bass as bass
import concourse.mybir as mybir
import concourse.tile as tile
from concourse.bass import IndirectOffsetOnAxis
from concourse.bass_utils import run_bass_kernel_spmd

B, C = 4096, 32000
NCORES = 8
R = B // NCORES          # rows per core: 512
P = 128                  # SBUF partitions
G = R // P               # row groups per core: 4
W = 8000                 # streaming column-tile width

# Group 3 tapers gradually: the in-order ACT engine trails the DMA stream
# by one full-tile EXP (~7.2us); an abrupt taper lets DMA sprint through
# the small tiles while ACT drains that backlog, so ACT (not DMA) would end
# the kernel. This profile (from a calibrated lag-recurrence search) hands
# ACT steadily shrinking tiles so it finishes ~1.3us after the last byte.
FULL = [W, W, W, W]                                        # groups 0-2
TAPER = [7625, 6000, 4250, 2375, 2250, 2125, 2125, 1500, 1500, 1250, 1000]
WIDTHS = [FULL, FULL, FULL, TAPER]

# Scheduler hint (ms of modeled time): floor for the small-exp chain's
# placement. See the tile_wait_until note in _kernel_body.
SMALL_CHAIN_WAIT_MS = 0.15
assert all(sum(ws) == C for ws in WIDTHS)
NCOLS = sum(len(ws) for ws in WIDTHS)

f32 = mybir.dt.float32
i32 = mybir.dt.int32
Alu = mybir.AluOpType
Act = mybir.ActivationFunctionType
AX = mybir.AxisListType.X


def _build_kernel() -> bass.Bass:
    nc = bacc.Bacc()
    x = nc.declare_dram_parameter("x", [R, C], f32, isOutput=False)
    tgt = nc.declare_dram_parameter("tgt", [P, G], i32, isOutput=False)
    tblf = nc.declare_dram_parameter("tblf", [C, 2], f32, isOutput=False)  # X1|X2
    tbli = nc.declare_dram_parameter("tbli", [C, 2], i32, isOutput=False)  # Y1|Y2
    tval = nc.declare_dram_parameter("tval", [P, 1], f32, isOutput=False)
    out = nc.declare_dram_parameter("out", [P, 4], f32, isOutput=True)

    with tile.TileContext(nc) as tc:
        _kernel_body(tc, x, tgt, tblf, tbli, tval, out)
    nc.compile()
    _merge_act_table_loads(nc)
    return nc


def _merge_act_table_loads(nc):
    """The auto-inserted ACT table loads pick exp_and_others then
    natural_log, paying a ~2.7us table switch right in the kernel tail.
    Set 6 (natural_log_exp_and_others) contains Exp, Ln, Identity and Copy,
    so point the first load at it and drop the later ones (they carry no
    sync)."""
    loads = [
        inst
        for f in nc.m.functions
        for blk in f.blocks
        for inst in blk.instructions
        if isinstance(inst, mybir.InstLoadActFuncSet)
    ]
    if any(inst.sync_info is not None for inst in loads):
        return  # unexpected shape; leave the program untouched
    first = True
    for f in nc.m.functions:
        for blk in f.blocks:
            keep = []
            for inst in blk.instructions:
                if isinstance(inst, mybir.InstLoadActFuncSet):
                    if first:
                        inst.act_func_set_id = 6
                        first = False
                    else:
                        continue
                keep.append(inst)
            if len(keep) != len(blk.instructions):
                blk.instructions[:] = keep


def _kernel_body(tc, x, tgt, tblf, tbli, tval, out):
    nc = tc.nc
    with (
        tc.tile_pool(name="const", bufs=1) as const,
        tc.tile_pool(name="stream", bufs=6) as stream,
        tc.tile_pool(name="small", bufs=1) as small,
    ):
        stats = const.tile([P, NCOLS], f32)
        # Explicit zero-bias tile for every activation: a float bias would
        # force a const-AP tensor whose TENSOR_LOAD sits on the Sync queue
        # ahead of the first stream DMA (~1.2us of dead start time). Its
        # memset must be EMITTED before the first EXP that reads it (the
        # dependency tracker orders by emission, a later write would be WAR).
        zbias = const.tile([P, 1], f32)
        nc.vector.memset(zbias[:], 0.0)
        ones = const.tile([P, G], f32)
        nc.vector.memset(ones[:], 1.0)

        def stream_tile(g, coff, w, col):
            xt = stream.tile([P, W], f32, tag="xt")
            nc.sync.dma_start(out=xt[:, :w], in_=x[g * P:(g + 1) * P, coff:coff + w])
            # exp in place; only the fused row-sum (accum_out) is kept.
            nc.scalar.activation(
                out=xt[:, :w], in_=xt[:, :w], func=Act.Exp,
                bias=zbias[:, 0:1],
                accum_out=stats[:, col:col + 1])

        col_of = {}
        col = 0
        for g in range(G):
            for ti in range(len(WIDTHS[g])):
                col_of[(g, ti)] = col
                col += 1

        # ---- head: first two stream tiles before the small loads so the
        # memory-bound stream is first in the Sync queue -------------------
        coff = 0
        for ti in range(2):
            stream_tile(0, coff, WIDTHS[0][ti], col_of[(0, ti)])
            coff += WIDTHS[0][ti]

        # ---- small loads (on the GPSIMD DMA queue so they don't delay the
        # stream DMAs queued on Sync) ---------------------------------------
        t_tile = const.tile([P, G], i32)      # t_tile[p, g] = target[g*128 + p]
        nc.gpsimd.dma_start(out=t_tile[:], in_=tgt[:, :])
        tv = const.tile([P, 1], f32)
        nc.gpsimd.dma_start(out=tv[:], in_=tval[:, :])

        # ---- table gathers: row t of [C, 2] tables, per batch row --------
        # HW indirect DMA honors one offset per partition, so gather each
        # row-group (one [P, 1] offset column) separately.
        x1 = small.tile([P, G], f32)
        x2 = small.tile([P, G], f32)
        y1 = small.tile([P, G], i32)
        y2 = small.tile([P, G], i32)
        for g in range(G):
            xg = small.tile([P, 2], f32, tag=f"xg{g}")  # (X1[t], X2[t])
            nc.gpsimd.indirect_dma_start(
                out=xg[:], out_offset=None, in_=tblf[:, :],
                in_offset=IndirectOffsetOnAxis(ap=t_tile[:, g:g + 1], axis=0),
            )
            yg = small.tile([P, 2], i32, tag=f"yg{g}")  # (Y1[t], Y2[t])
            nc.gpsimd.indirect_dma_start(
                out=yg[:], out_offset=None, in_=tbli[:, :],
                in_offset=IndirectOffsetOnAxis(ap=t_tile[:, g:g + 1], axis=0),
            )
            nc.vector.tensor_copy(out=x1[:, g:g + 1], in_=xg[:, 0:1])
            nc.vector.tensor_copy(out=x2[:, g:g + 1], in_=xg[:, 1:2])
            nc.vector.tensor_copy(out=y1[:, g:g + 1], in_=yg[:, 0:1])
            nc.vector.tensor_copy(out=y2[:, g:g + 1], in_=yg[:, 1:2])

        # ---- flat element offsets into x for the 3 logit gathers ---------
        ridx = const.tile([P, G], i32)        # ridx[p, g] = g*128 + p
        nc.gpsimd.iota(out=ridx[:], pattern=[[P, G]], base=0,
                       channel_multiplier=1)
        rb = const.tile([P, G], i32)          # rb[p, g] = (g*128 + p) * C
        nc.vector.tensor_scalar(out=rb[:], in0=ridx[:], scalar1=C,
                                scalar2=None, op0=Alu.mult)
        off_t = small.tile([P, G], i32)
        nc.vector.tensor_tensor(out=off_t[:], in0=rb[:], in1=t_tile[:], op=Alu.add)
        off_1 = small.tile([P, G], i32)
        nc.vector.tensor_tensor(out=off_1[:], in0=rb[:], in1=y1[:], op=Alu.add)
        off_2 = small.tile([P, G], i32)
        nc.vector.tensor_tensor(out=off_2[:], in0=rb[:], in1=y2[:], op=Alu.add)

        xap = x[:, :]
        xflat = bass.AP(tensor=xap.tensor, offset=0, ap=[[1, R * C], [1, 1]])
        g_t = small.tile([P, G], f32)         # logits[i, t]
        g_1 = small.tile([P, G], f32)         # logits[i, Y1[t]]
        g_2 = small.tile([P, G], f32)         # logits[i, Y2[t]]
        for g in range(G):
            for dst, off in ((g_t, off_t), (g_1, off_1), (g_2, off_2)):
                nc.gpsimd.indirect_dma_start(
                    out=dst[:, g:g + 1], out_offset=None, in_=xflat,
                    in_offset=IndirectOffsetOnAxis(ap=off[:, g:g + 1], axis=0),
                )

        # ---- groups 0-2 of the stream ------------------------------------
        S = small.tile([P, G], f32)           # S[p, g] = row sum of exp
        lnS = small.tile([P, G], f32)

        def emit_group(g, skip=0):
            coff = sum(WIDTHS[g][:skip])
            for ti in range(skip, len(WIDTHS[g])):
                stream_tile(g, coff, WIDTHS[g][ti], col_of[(g, ti)])
                coff += WIDTHS[g][ti]

        def reduce_group(g):
            c0 = col_of[(g, 0)]
            c1 = col_of[(g, len(WIDTHS[g]) - 1)] + 1
            c = slice(g, g + 1)
            nc.vector.tensor_reduce(
                out=S[:, c], in_=stats[:, c0:c1], axis=AX, op=Alu.add)
            nc.scalar.activation(out=lnS[:, c], in_=S[:, c], func=Act.Ln,
                                 bias=zbias[:, 0:1])

        emit_group(0, skip=2)
        reduce_group(0)
        emit_group(1)
        reduce_group(1)
        emit_group(2)
        reduce_group(2)

        # ---- S-independent per-row math ----------------------------------
        # Wrapped in tile_wait_until so the scheduler places this whole
        # gather-dependent chain LATE in the ACT and DVE queues (its
        # criticality heuristic otherwise schedules the small exps before
        # the first stream EXP, and the in-order queues then stall ~40us
        # waiting on the indirect gathers).
        ctx_wait = tc.tile_wait_until(SMALL_CHAIN_WAIT_MS)
        ctx_wait.__enter__()
        # The row-sum S only scales p_t/P1/P2 uniformly, so every
        # comparison and ratio can be computed from the raw exp'd logits:
        #   cond:  p_t > corr      <=>  e_t > cnum,  cnum = T*(x1*e1 + x2*e2)
        #   z:     p_t / corr       =   e_t / cnum
        #   nz:    P1 != 0 or P2 != 0  <=>  e_1 != 0 or e_2 != 0
        # Only the loss term needs S, and it splits as
        #   -log(d_pre / S) = log(S) - log(d_pre),
        #   d_pre = (e_t - cnum) if cond else e_t,
        # so log(d_pre) is computed early too; only log(S) trails the
        # stream.
        e_t = small.tile([P, G], f32)
        nc.scalar.activation(out=e_t[:], in_=g_t[:], func=Act.Exp, bias=zbias[:, 0:1])
        e_1 = small.tile([P, G], f32)
        nc.scalar.activation(out=e_1[:], in_=g_1[:], func=Act.Exp, bias=zbias[:, 0:1])
        e_2 = small.tile([P, G], f32)
        nc.scalar.activation(out=e_2[:], in_=g_2[:], func=Act.Exp, bias=zbias[:, 0:1])
        a = small.tile([P, G], f32)
        nc.vector.tensor_tensor(out=a[:], in0=x1[:], in1=e_1[:], op=Alu.mult)
        b = small.tile([P, G], f32)
        nc.vector.tensor_tensor(out=b[:], in0=x2[:], in1=e_2[:], op=Alu.mult)
        s = small.tile([P, G], f32)
        nc.vector.tensor_tensor(out=s[:], in0=a[:], in1=b[:], op=Alu.add)
        cnum = small.tile([P, G], f32)        # corr * S
        nc.vector.tensor_scalar(out=cnum[:], in0=s[:], scalar1=tv[:, 0:1],
                                scalar2=None, op0=Alu.mult)
        cond_i = small.tile([P, G], i32)      # 1 where p_t > corr (int mask)
        nc.vector.tensor_tensor(out=cond_i[:], in0=e_t[:], in1=cnum[:], op=Alu.is_gt)
        cond = small.tile([P, G], f32)
        nc.vector.tensor_copy(out=cond[:], in_=cond_i[:])
        diff = small.tile([P, G], f32)
        nc.vector.tensor_tensor(out=diff[:], in0=e_t[:], in1=cnum[:], op=Alu.subtract)
        d_pre = small.tile([P, G], f32)
        nc.vector.select(out=d_pre[:], mask=cond_i[:], on_true=diff[:], on_false=e_t[:])
        nz1 = small.tile([P, G], i32)
        nc.vector.tensor_scalar(out=nz1[:], in0=e_1[:], scalar1=0.0,
                                scalar2=None, op0=Alu.not_equal)
        nz2 = small.tile([P, G], i32)
        nc.vector.tensor_scalar(out=nz2[:], in0=e_2[:], scalar1=0.0,
                                scalar2=None, op0=Alu.not_equal)
        nz = small.tile([P, G], i32)
        nc.vector.tensor_tensor(out=nz[:], in0=nz1[:], in1=nz2[:], op=Alu.bitwise_or)
        k_i = small.tile([P, G], i32)         # cond and nz (int mask)
        nc.vector.tensor_tensor(out=k_i[:], in0=cond_i[:], in1=nz[:], op=Alu.bitwise_and)
        k = small.tile([P, G], f32)
        nc.vector.tensor_copy(out=k[:], in_=k_i[:])
        safe = small.tile([P, G], f32)        # cnum where k else 1.0
        nc.vector.select(out=safe[:], mask=k_i[:], on_true=cnum[:], on_false=ones[:])
        rsafe = small.tile([P, G], f32)
        nc.vector.reciprocal(out=rsafe[:], in_=safe[:])
        z0 = small.tile([P, G], f32)
        nc.vector.tensor_tensor(out=z0[:], in0=e_t[:], in1=rsafe[:], op=Alu.mult)
        z = small.tile([P, G], f32)
        nc.vector.tensor_tensor(out=z[:], in0=z0[:], in1=k[:], op=Alu.mult)
        j = small.tile([P, G], f32)           # 1 - cond
        nc.vector.tensor_scalar(out=j[:], in0=cond[:], scalar1=-1.0,
                                scalar2=1.0, op0=Alu.mult, op1=Alu.add)
        lnd_pre = small.tile([P, G], f32)
        nc.scalar.activation(out=lnd_pre[:], in_=d_pre[:], func=Act.Ln, bias=zbias[:, 0:1])
        Qd = small.tile([P, 1], f32)          # per-partition sum of ln(d_pre)
        nc.vector.tensor_reduce(out=Qd[:], in_=lnd_pre[:], axis=AX, op=Alu.add)
        Q = small.tile([P, 4], f32)
        nc.vector.tensor_reduce(out=Q[:, 1:2], in_=k[:], axis=AX, op=Alu.add)
        nc.vector.tensor_reduce(out=Q[:, 2:3], in_=z[:], axis=AX, op=Alu.add)
        nc.vector.tensor_reduce(out=Q[:, 3:4], in_=j[:], axis=AX, op=Alu.add)
        # fold groups 0-2's ln(S) into the running loss partial
        t01 = small.tile([P, 1], f32)         # lnS0 + lnS1
        nc.vector.tensor_tensor(out=t01[:], in0=lnS[:, 0:1], in1=lnS[:, 1:2],
                                op=Alu.add)
        t012 = small.tile([P, 1], f32)
        nc.vector.tensor_tensor(out=t012[:], in0=t01[:], in1=lnS[:, 2:3],
                                op=Alu.add)
        Qpre = small.tile([P, 1], f32)        # Qd - (lnS0 + lnS1 + lnS2)
        nc.vector.tensor_tensor(out=Qpre[:], in0=Qd[:], in1=t012[:],
                                op=Alu.subtract)
        ctx_wait.__exit__(None, None, None)

        # ---- group 3 (tapered) and the all-ACT tail ----------------------
        n3 = len(WIDTHS[3])
        coff = 0
        for ti in range(n3 - 1):
            stream_tile(3, coff, WIDTHS[3][ti], col_of[(3, ti)])
            coff += WIDTHS[3][ti]
        c0 = col_of[(3, 0)]
        clast = col_of[(3, n3 - 1)]
        S3a = small.tile([P, 1], f32)         # partial S3 over all but last tile
        nc.vector.tensor_reduce(out=S3a[:], in_=stats[:, c0:clast], axis=AX,
                                op=Alu.add)
        stream_tile(3, coff, WIDTHS[3][n3 - 1], clast)
        # Tail runs entirely on ACT (Identity-with-AP-bias), no DVE hops:
        #   S3 = stats[last] + S3a; lnS3 = ln(S3); Q0 = Qpre - lnS3.
        nc.scalar.activation(out=S[:, 3:4], in_=stats[:, clast:clast + 1],
                             func=Act.Identity, bias=S3a[:, 0:1])
        nc.scalar.activation(out=lnS[:, 3:4], in_=S[:, 3:4], func=Act.Ln,
                             bias=zbias[:, 0:1])
        nc.scalar.activation(out=Q[:, 0:1], in_=lnS[:, 3:4],
                             func=Act.Identity, bias=Qpre[:, 0:1], scale=-1.0)

        # ---- per-partition partials out; host sums the 128 lanes ---------
        nc.sync.dma_start(out=out[:, :], in_=Q[:])


_NC_CACHE = None


def _get_nc() -> bass.Bass:
    global _NC_CACHE
    if _NC_CACHE is None:
        _NC_CACHE = _build_kernel()
    return _NC_CACHE


def make_in_maps(input, target, X1, Y1, X2, Y2, T):
    """Shard the full inputs into per-core input maps."""
    input = np.ascontiguousarray(np.asarray(input, dtype=np.float32))
    target = np.asarray(target).astype(np.int32)
    tblf = np.ascontiguousarray(
        np.stack([np.asarray(X1, np.float32)[0], np.asarray(X2, np.float32)[0]],
                 axis=1))
    tbli = np.ascontiguousarray(
        np.stack([np.asarray(Y1)[0].astype(np.int32),
                  np.asarray(Y2)[0].astype(np.int32)], axis=1))
    tval = np.full((P, 1), np.asarray(T, np.float32)[0], dtype=np.float32)

    in_maps = []
    for c in range(NCORES):
        tg = target[c * R:(c + 1) * R].reshape(G, P).T  # [P, G]
        in_maps.append({
            "x": np.ascontiguousarray(input[c * R:(c + 1) * R]),
            "tgt": np.ascontiguousarray(tg),
            "tblf": tblf,
            "tbli": tbli,
            "tval": tval,
        })
    return in_maps


def combine_outputs(results):
    """Sum the per-core, per-partition [128, 4] partials on the host."""
    outs = np.stack([np.asarray(r["out"]) for r in results])  # [ncores, P, 4]
    tot = outs.sum(axis=(0, 1), dtype=np.float64)
    loss = np.float32(-tot[0] / B)
    return (loss, np.float32(tot[1]), np.float32(tot[2]), np.float32(tot[3]))


def kernel(input, target, X1, Y1, X2, Y2, T):
    nc = _get_nc()
    in_maps = make_in_maps(input, target, X1, Y1, X2, Y2, T)
    res = run_bass_kernel_spmd(nc, in_maps, core_ids=list(range(NCORES)))
    return combine_outputs(res.results)
